# revision 6
# baseline (speedup 1.0000x reference)
"""DANet spatial-attention (SAModule) Trainium2 kernel.

Shards B=16 across 8 NeuronCores (2 batches/core). Per batch, on one core:
  qk proj -> S^T = k^T q -> exp -> colsum (ones-matmul) -> out = v^T.T @ expT
  -> epilogue out * (gamma/colsum) + x, all in a transposed layout so no
  on-chip transposes are needed. Matmuls in bf16, residual/output f32-ish.
"""
import sys
sys.path.insert(0, '/opt/trn_rl_repo')

import numpy as np
import ml_dtypes

import bass_rust
import concourse.bass as bass
import concourse.tile as tile
from concourse import mybir
from concourse.bass_utils import run_bass_kernel_spmd

# ---------------------------------------------------------------------------
# Workaround: this walrus build rejects >1 sync-wait on the Tile tail drain.
# Split the accumulated waits across a chain of drains, one wait each.
def _patched_drain_and_barrier(self, tick_clock, wait_clock):
    nc = self.nc
    drain_inst = nc.sync.drain()
    wait_clock.add_sem_waits(
        drain_inst.ins, tile.ScopedClock({None: tick_clock.global_clock})
    )
    si = drain_inst.ins.sync_info
    if si is not None and len(si.on_wait) > 1:
        extra = list(si.on_wait[1:])
        si.on_wait = si.on_wait[:1]
        for w in extra:
            d2 = nc.sync.drain()
            d2.ins.sync_info = bass_rust.SyncInfo(on_wait=[w], on_update=[])
    nc.all_engine_barrier()
    assert self.sems is not None
    popped = nc._tile_sem_poison_stack.pop()
    assert popped is self._sem_poison
    nc.clear_and_free_semaphores(list(self.sems.allocated().values()))
    nc.all_engine_barrier()

tile.TileContext._drain_and_barrier = _patched_drain_and_barrier


def split_multi_waits(nc):
    """This walrus build accepts at most one sync-wait per instruction.
    Hoist extra waits onto same-engine NOPs inserted just before."""
    for f in nc.m.functions:
        for bb in f.blocks:
            insts = bb.instructions
            out = []
            for ins in insts:
                si = ins.sync_info
                if si is not None and len(si.on_wait) > 1:
                    extra = list(si.on_wait[1:])
                    si.on_wait = si.on_wait[:1]
                    for w in extra:
                        nop = mybir.InstNoOp(
                            name=nc.get_next_instruction_name(),
                            engine=ins.engine,
                            sync_info=bass_rust.SyncInfo(
                                on_wait=[w], on_update=[]),
                            bass_nofuse=True,
                        )
                        nc.register_instruction(nop, overwrite=True)
                        out.append(nop)
                out.append(ins)
            if len(out) != len(insts):
                insts[:] = out
# ---------------------------------------------------------------------------

B, C, H, W = 16, 512, 48, 48
HW = H * W                     # 2304
D = 64
NCORES = 8
BPC = B // NCORES              # 2 batches per core
CI = C // 128                  # 4 c-chunks
MC = HW // 128                 # 18 m-chunks
N_BLOCKS = [(0, 512), (512, 512), (1024, 512), (1536, 512), (2048, 256)]

BF16 = mybir.dt.bfloat16
F32 = mybir.dt.float32
AF = mybir.ActivationFunctionType

PACK_T = True   # 2x row-tiled packing of the K=64 S^T matmuls


def build_bass():
    nc = bass.Bass()
    x_d = nc.dram_tensor("x", [BPC, 128, CI, HW], BF16, kind="ExternalInput")
    wqq_d = nc.dram_tensor("wqq", [128, CI, 128], BF16, kind="ExternalInput")
    wkk_d = nc.dram_tensor("wkk", [128, CI, 128], BF16, kind="ExternalInput")
    wv_d = nc.dram_tensor("wv", [128, CI, 512], BF16, kind="ExternalInput")
    bqq_d = nc.dram_tensor("bqq", [128, 1], F32, kind="ExternalInput")
    bkk_d = nc.dram_tensor("bkk", [128, 1], F32, kind="ExternalInput")
    bvr_d = nc.dram_tensor("bvr", [128, 512], F32, kind="ExternalInput")
    onec_d = nc.dram_tensor("onec", [128, 1], BF16, kind="ExternalInput")
    oner_d = nc.dram_tensor("oner", [1, 128], BF16, kind="ExternalInput")
    gamma_d = nc.dram_tensor("gamma", [1, 1], F32, kind="ExternalInput")
    out_d = nc.dram_tensor("out", [BPC, C, HW], F32, kind="ExternalOutput")

    with tile.TileContext(nc) as tc:
        with (
            tc.tile_pool(name="singles", bufs=1) as singles,
            tc.tile_pool(name="xpool", bufs=2) as xpool,
            tc.tile_pool(name="big", bufs=1) as big,
            tc.tile_pool(name="small", bufs=3) as small,
            tc.tile_pool(name="ytile", bufs=3) as ytile,
        ):
            wqq = singles.tile([128, CI, 128], BF16)
            nc.sync.dma_start(out=wqq, in_=wqq_d[:])
            wkk = singles.tile([128, CI, 128], BF16)
            nc.sync.dma_start(out=wkk, in_=wkk_d[:])
            wv = singles.tile([128, CI, 512], BF16)
            nc.sync.dma_start(out=wv, in_=wv_d[:])
            bqq = singles.tile([128, 1], F32)
            nc.sync.dma_start(out=bqq, in_=bqq_d[:])
            bkk = singles.tile([128, 1], F32)
            nc.sync.dma_start(out=bkk, in_=bkk_d[:])
            bvr = singles.tile([128, 512], F32)
            nc.sync.dma_start(out=bvr, in_=bvr_d[:])
            onec = singles.tile([128, 1], BF16)
            nc.sync.dma_start(out=onec, in_=onec_d[:])
            oner = singles.tile([1, 128], BF16)
            nc.sync.dma_start(out=oner, in_=oner_d[:])
            gam = singles.tile([1, 1], F32)
            nc.sync.dma_start(out=gam, in_=gamma_d[:])

            for b in range(BPC):
                x_sb = xpool.tile([128, CI, HW], BF16, tag="x", name="x")
                nc.sync.dma_start(out=x_sb, in_=x_d[b])

                qq_sb = big.tile([128, HW], BF16, tag="qq", name="qq")
                kk_sb = big.tile([128, HW], BF16, tag="kk", name="kk")
                vt_sb = big.tile([128, MC, 512], BF16, tag="vt", name="vt")
                expt_sb = big.tile([128, MC, HW], BF16, tag="expt", name="expt")
                rep_sb = big.tile([128, HW], F32, tag="rep", name="rep_sb")

                # ---- Phase A: qq/kk projections ([q;q], [k;k] stacked) ----
                with tc.tile_pool(name="proj_ps", bufs=3, space="PSUM") as pps:
                    for lo, w in N_BLOCKS:
                        for wt, bias, dst in ((wqq, bqq, qq_sb), (wkk, bkk, kk_sb)):
                            ps = pps.tile([128, 512], F32, tag="proj", name="proj")[:, :w]
                            for ci in range(CI):
                                nc.tensor.matmul(
                                    ps, lhsT=wt[:, ci, :], rhs=x_sb[:, ci, lo:lo + w],
                                    start=(ci == 0), stop=(ci == CI - 1))
                            nc.scalar.activation(
                                out=dst[:, lo:lo + w], in_=ps, func=AF.Identity,
                                bias=bias, scale=1.0)

                    # ---- Phase B: vT[m, c] = x^T @ Wv^T + bv ----
                    for m in range(MC):
                        ps = pps.tile([128, 512], F32, tag="vt_ps", name="vt_ps")
                        for ci in range(CI):
                            nc.tensor.matmul(
                                ps, lhsT=x_sb[:, ci, m * 128:(m + 1) * 128],
                                rhs=wv[:, ci, :],
                                start=(ci == 0), stop=(ci == CI - 1))
                        nc.vector.tensor_add(out=vt_sb[:, m, :], in0=ps, in1=bvr)

                # ---- Phase C+D: S^T, exp, column sums, gamma/colsum rep ----
                with (
                    tc.tile_pool(name="t_ps", bufs=4, space="PSUM") as tps,
                    tc.tile_pool(name="cs_ps", bufs=2, space="PSUM") as csps,
                    tc.tile_pool(name="rep_ps", bufs=2, space="PSUM") as repps,
                ):
                    for lo, w in N_BLOCKS:
                        cs = csps.tile([1, 512], F32, tag="cs", name="cs")[:, :w]
                        if PACK_T:
                            for m2 in range(MC // 2):
                                ps0 = tps.tile([128, 512], F32, tag="t", name="t")[:, :w]
                                ps1 = tps.tile([128, 512], F32, tag="t", name="t")[:, :w]
                                m0, m1 = 2 * m2, 2 * m2 + 1
                                nc.tensor.matmul(
                                    ps0, lhsT=kk_sb[0:64, m0 * 128:(m0 + 1) * 128],
                                    rhs=qq_sb[0:64, lo:lo + w],
                                    start=True, stop=True, tile_position=(0, 0))
                                nc.tensor.matmul(
                                    ps1, lhsT=kk_sb[64:128, m1 * 128:(m1 + 1) * 128],
                                    rhs=qq_sb[64:128, lo:lo + w],
                                    start=True, stop=True, tile_position=(64, 0))
                                for m, ps in ((m0, ps0), (m1, ps1)):
                                    nc.scalar.activation(
                                        out=expt_sb[:, m, lo:lo + w], in_=ps, func=AF.Exp)
                                    nc.tensor.matmul(
                                        cs, lhsT=onec, rhs=expt_sb[:, m, lo:lo + w],
                                        start=(m == 0), stop=(m == MC - 1))
                        else:
                            for m in range(MC):
                                ps = tps.tile([128, 512], F32, tag="t", name="t")[:, :w]
                                nc.tensor.matmul(
                                    ps, lhsT=kk_sb[0:64, m * 128:(m + 1) * 128],
                                    rhs=qq_sb[0:64, lo:lo + w],
                                    start=True, stop=True)
                                nc.scalar.activation(
                                    out=expt_sb[:, m, lo:lo + w], in_=ps, func=AF.Exp)
                                nc.tensor.matmul(
                                    cs, lhsT=onec, rhs=expt_sb[:, m, lo:lo + w],
                                    start=(m == 0), stop=(m == MC - 1))

                        recf = small.tile([1, 512], F32, tag="recf", name="recf")[:, :w]
                        nc.vector.reciprocal(out=recf, in_=cs)
                        recb = small.tile([1, 512], BF16, tag="recb", name="recb")[:, :w]
                        nc.vector.tensor_scalar_mul(out=recb, in0=recf, scalar1=gam)
                        rp = repps.tile([128, 512], F32, tag="rep", name="rep")[:, :w]
                        nc.tensor.matmul(rp, lhsT=oner, rhs=recb, start=True, stop=True)
                        nc.vector.tensor_copy(out=rep_sb[:, lo:lo + w], in_=rp)

                # ---- Phase E: out[c, n] = vT.T @ expT; epilogue + residual ----
                with tc.tile_pool(name="out_ps", bufs=1, space="PSUM") as ops:
                    for ci in range(CI):
                        pss = [
                            ops.tile([128, 512], F32, tag=f"out{k}", name=f"out{k}")[:, :w]
                            for k, (lo, w) in enumerate(N_BLOCKS)
                        ]
                        for m in range(MC):
                            for k, (lo, w) in enumerate(N_BLOCKS):
                                nc.tensor.matmul(
                                    pss[k], lhsT=vt_sb[:, m, ci * 128:(ci + 1) * 128],
                                    rhs=expt_sb[:, m, lo:lo + w],
                                    start=(m == 0), stop=(m == MC - 1))
                        for k, (lo, w) in enumerate(N_BLOCKS):
                            y = ytile.tile([128, 512], F32, tag="y", name="y")[:, :w]
                            nc.vector.tensor_mul(
                                out=y, in0=pss[k], in1=rep_sb[:, lo:lo + w])
                            nc.vector.tensor_add(
                                out=y, in0=y, in1=x_sb[:, ci, lo:lo + w])
                            nc.sync.dma_start(
                                out=out_d[b, ci * 128:(ci + 1) * 128, lo:lo + w], in_=y)
    split_multi_waits(nc)
    return nc


_NC_CACHE = {}


def _get_nc():
    if "nc" not in _NC_CACHE:
        _NC_CACHE["nc"] = build_bass()
    return _NC_CACHE["nc"]


def _prepare_in_maps(feat_map, Wq, bq, Wk, bk, Wv, bv, gamma):
    feat_map = np.asarray(feat_map, dtype=np.float32)
    Wq = np.asarray(Wq, dtype=np.float32)
    bq = np.asarray(bq, dtype=np.float32)
    Wk = np.asarray(Wk, dtype=np.float32)
    bk = np.asarray(bk, dtype=np.float32)
    Wv = np.asarray(Wv, dtype=np.float32)
    bv = np.asarray(bv, dtype=np.float32)
    gamma = np.asarray(gamma, dtype=np.float32)

    bf = ml_dtypes.bfloat16
    # x: [B, C, HW] -> [B, 128, CI, HW] (partition = c % 128, chunk = c // 128)
    x = feat_map.reshape(B, CI, 128, HW).transpose(0, 2, 1, 3).astype(bf)
    x = np.ascontiguousarray(x)

    def prep_w(Wt2):  # [C, M] -> [128, CI, M]
        M = Wt2.shape[1]
        return np.ascontiguousarray(
            Wt2.reshape(CI, 128, M).transpose(1, 0, 2).astype(bf))

    wqq = prep_w(np.concatenate([Wq.T, Wq.T], axis=1))   # [512, 128]
    wkk = prep_w(np.concatenate([Wk.T, Wk.T], axis=1))
    wv = prep_w(Wv.T)                                     # [512, 512]
    bqq = np.concatenate([bq, bq]).reshape(128, 1).astype(np.float32)
    bkk = np.concatenate([bk, bk]).reshape(128, 1).astype(np.float32)
    bvr = np.ascontiguousarray(np.tile(bv[None, :], (128, 1)).astype(np.float32))
    onec = np.ones((128, 1), dtype=bf)
    oner = np.ones((1, 128), dtype=bf)
    gam = gamma.reshape(1, 1).astype(np.float32)

    in_maps = []
    for i in range(NCORES):
        in_maps.append({
            "x": np.ascontiguousarray(x[i * BPC:(i + 1) * BPC]),
            "wqq": wqq, "wkk": wkk, "wv": wv,
            "bqq": bqq, "bkk": bkk, "bvr": bvr,
            "onec": onec, "oner": oner, "gamma": gam,
        })
    return in_maps


def _run(in_maps, trace=False, **kw):
    nc = _get_nc()
    res = run_bass_kernel_spmd(nc, in_maps, core_ids=list(range(NCORES)),
                               trace=trace, **kw)
    out = np.concatenate([res.results[i]["out"] for i in range(NCORES)], axis=0)
    return out.reshape(B, C, H, W).astype(np.float32), res


def kernel(feat_map, Wq, bq, Wk, bk, Wv, bv, gamma):
    in_maps = _prepare_in_maps(feat_map, Wq, bq, Wk, bk, Wv, bv, gamma)
    out, _ = _run(in_maps, trace=False)
    return out


# revision 14
# speedup vs baseline: 1.6198x; 1.6198x over previous
"""DANet spatial-attention (SAModule) Trainium2 kernel.

Shards B=16 across 8 NeuronCores (2 batches/core). Per batch, on one core:
  qk proj -> S^T = k^T q -> exp -> colsum (ones-matmul) -> out = v^T.T @ expT
  -> epilogue out * (gamma/colsum) + x, all in a transposed layout so no
  on-chip transposes are needed. Matmuls in bf16, residual/output f32-ish.
"""
import sys
sys.path.insert(0, '/opt/trn_rl_repo')

import numpy as np
import ml_dtypes

import bass_rust
import concourse.bass as bass
import concourse.tile as tile
from concourse import mybir
from concourse.bass_utils import run_bass_kernel_spmd

# ---------------------------------------------------------------------------
# Workaround: this walrus build rejects >1 sync-wait on the Tile tail drain.
# Split the accumulated waits across a chain of drains, one wait each.
def _patched_drain_and_barrier(self, tick_clock, wait_clock):
    nc = self.nc
    drain_inst = nc.sync.drain()
    wait_clock.add_sem_waits(
        drain_inst.ins, tile.ScopedClock({None: tick_clock.global_clock})
    )
    si = drain_inst.ins.sync_info
    if si is not None and len(si.on_wait) > 1:
        extra = list(si.on_wait[1:])
        si.on_wait = si.on_wait[:1]
        for w in extra:
            d2 = nc.sync.drain()
            d2.ins.sync_info = bass_rust.SyncInfo(on_wait=[w], on_update=[])
    nc.all_engine_barrier()
    assert self.sems is not None
    popped = nc._tile_sem_poison_stack.pop()
    assert popped is self._sem_poison
    nc.clear_and_free_semaphores(list(self.sems.allocated().values()))
    nc.all_engine_barrier()

tile.TileContext._drain_and_barrier = _patched_drain_and_barrier


def split_multi_waits(nc):
    """This walrus build accepts at most one sync-wait per instruction.
    Hoist extra waits onto same-engine NOPs inserted just before."""
    for f in nc.m.functions:
        for bb in f.blocks:
            insts = bb.instructions
            out = []
            for ins in insts:
                si = ins.sync_info
                if si is not None and len(si.on_wait) > 1:
                    extra = list(si.on_wait[1:])
                    si.on_wait = si.on_wait[:1]
                    for w in extra:
                        nop = mybir.InstNoOp(
                            name=nc.get_next_instruction_name(),
                            engine=ins.engine,
                            sync_info=bass_rust.SyncInfo(
                                on_wait=[w], on_update=[]),
                            bass_nofuse=True,
                        )
                        nc.register_instruction(nop, overwrite=True)
                        out.append(nop)
                out.append(ins)
            if len(out) != len(insts):
                insts[:] = out
# ---------------------------------------------------------------------------

B, C, H, W = 16, 512, 48, 48
HW = H * W                     # 2304
D = 64
NCORES = 8
BPC = B // NCORES              # 2 batches per core
CI = C // 128                  # 4 c-chunks
MC = HW // 128                 # 18 m-chunks
N_BLOCKS = [(0, 512), (512, 512), (1024, 512), (1536, 512), (2048, 256)]

BF16 = mybir.dt.bfloat16
FP8 = mybir.dt.float8e4
F32 = mybir.dt.float32
AF = mybir.ActivationFunctionType

PACK_T = True   # 2x row-tiled packing of the K=64 S^T matmuls


def build_bass():
    nc = bass.Bass()
    x_d = nc.dram_tensor("x", [BPC, 128, CI, HW], BF16, kind="ExternalInput")
    wqq_d = nc.dram_tensor("wqq", [128, CI, 128], BF16, kind="ExternalInput")
    wkk_d = nc.dram_tensor("wkk", [128, CI, 128], BF16, kind="ExternalInput")
    wv_d = nc.dram_tensor("wv", [128, CI, 512], BF16, kind="ExternalInput")
    bqq_d = nc.dram_tensor("bqq", [128, 1], F32, kind="ExternalInput")
    bkk_d = nc.dram_tensor("bkk", [128, 1], F32, kind="ExternalInput")
    bvr_d = nc.dram_tensor("bvr", [128, 512], F32, kind="ExternalInput")
    onec_d = nc.dram_tensor("onec", [128, 1], FP8, kind="ExternalInput")
    oner_d = nc.dram_tensor("oner", [1, 128], BF16, kind="ExternalInput")
    gamma_d = nc.dram_tensor("gamma", [1, 1], F32, kind="ExternalInput")
    out_d = nc.dram_tensor("out", [BPC, C, HW], F32, kind="ExternalOutput")

    with tile.TileContext(nc) as tc:
        with (
            tc.tile_pool(name="singles", bufs=1) as singles,
            tc.tile_pool(name="xpool", bufs=2) as xpool,
            tc.tile_pool(name="big", bufs=1) as big,
            tc.tile_pool(name="small", bufs=3) as small,
            tc.tile_pool(name="ytile", bufs=3) as ytile,
        ):
            wqq = singles.tile([128, CI, 128], BF16)
            nc.gpsimd.dma_start(out=wqq, in_=wqq_d[:])
            wkk = singles.tile([128, CI, 128], BF16)
            nc.gpsimd.dma_start(out=wkk, in_=wkk_d[:])
            wv = singles.tile([128, CI, 512], BF16)
            nc.gpsimd.dma_start(out=wv, in_=wv_d[:])
            bqq = singles.tile([128, 1], F32)
            nc.gpsimd.dma_start(out=bqq, in_=bqq_d[:])
            bkk = singles.tile([128, 1], F32)
            nc.gpsimd.dma_start(out=bkk, in_=bkk_d[:])
            bvr = singles.tile([128, 512], F32)
            nc.gpsimd.dma_start(out=bvr, in_=bvr_d[:])
            onec = singles.tile([128, 1], FP8)
            nc.gpsimd.dma_start(out=onec, in_=onec_d[:])
            oner = singles.tile([1, 128], BF16)
            nc.gpsimd.dma_start(out=oner, in_=oner_d[:])
            gam = singles.tile([1, 1], F32)
            nc.gpsimd.dma_start(out=gam, in_=gamma_d[:])
            shift = singles.tile([128, 1], F32)
            nc.vector.memset(shift, -5.0)

            for b in range(BPC):
                x_sb = xpool.tile([128, CI, HW], BF16, tag="x", name="x")
                for ci in range(CI):
                    nc.gpsimd.dma_start(out=x_sb[:, ci, :], in_=x_d[b, :, ci, :])

                qq_sb = big.tile([128, HW], BF16, tag="qq", name="qq")
                kk_sb = big.tile([128, HW], BF16, tag="kk", name="kk")
                vt_sb = big.tile([128, MC, 512], FP8, tag="vt", name="vt")
                expt_sb = big.tile([128, MC, HW], FP8, tag="expt", name="expt")
                rep_sb = big.tile([128, HW], F32, tag="rep", name="rep_sb")

                with (
                    tc.tile_pool(name="proj_ps", bufs=2, space="PSUM") as pps,
                    tc.tile_pool(name="t_ps", bufs=2, space="PSUM") as tps,
                    tc.tile_pool(name="cs_ps", bufs=1, space="PSUM") as csps,
                    tc.tile_pool(name="rep_ps", bufs=1, space="PSUM") as repps,
                ):
                    # ---- Phase A: qq/kk projections ([q;q], [k;k] stacked) ----
                    for lo, w in N_BLOCKS:
                        for wt, bias, dst in ((wqq, bqq, qq_sb), (wkk, bkk, kk_sb)):
                            ps = pps.tile([128, 512], F32, tag="proj", name="proj")[:, :w]
                            for ci in range(CI):
                                nc.tensor.matmul(
                                    ps, lhsT=wt[:, ci, :], rhs=x_sb[:, ci, lo:lo + w],
                                    start=(ci == 0), stop=(ci == CI - 1))
                            nc.scalar.activation(
                                out=dst[:, lo:lo + w], in_=ps, func=AF.Identity,
                                bias=bias, scale=1.0)

                    # ---- Phase C+D fused with B: per n-block, software-
                    # pipelined T-pair -> exp(pair) -> colsum, with the vT
                    # projection interleaved to keep the PE dense while the
                    # ACT engine chews through the exps. ----
                    def emit_b_chunk(m):
                        ps = pps.tile([128, 512], F32, tag="proj", name="projb")
                        for ci in range(CI):
                            nc.tensor.matmul(
                                ps, lhsT=x_sb[:, ci, m * 128:(m + 1) * 128],
                                rhs=wv[:, ci, :],
                                start=(ci == 0), stop=(ci == CI - 1))
                        nc.vector.tensor_add(out=vt_sb[:, m, :], in0=ps, in1=bvr)

                    b_next = [0]

                    def maybe_b():
                        if b_next[0] < MC:
                            emit_b_chunk(b_next[0])
                            b_next[0] += 1

                    NPAIR = MC // 2

                    def emit_rep_chain(lo, w, cs):
                        # gamma/colsum, replicated to all 128 partitions via a
                        # K=1 matmul. Emitted lagged so the PE never waits on
                        # the DVE reciprocal chain.
                        recf = small.tile([1, 512], F32, tag="recf", name="recf")[:, :w]
                        nc.vector.reciprocal(out=recf, in_=cs)
                        recg = small.tile([1, 512], BF16, tag="recg", name="recg")[:, :w]
                        nc.vector.tensor_scalar_mul(out=recg, in0=recf, scalar1=gam)
                        rp = repps.tile([128, 512], F32, tag="rep", name="rep")[:, :w]
                        nc.tensor.matmul(rp, lhsT=oner, rhs=recg, start=True, stop=True)
                        nc.vector.tensor_copy(out=rep_sb[:, lo:lo + w], in_=rp)

                    pending_rep = None
                    step = 0
                    for bi, (lo, w) in enumerate(N_BLOCKS):
                        cs = csps.tile([1, 512], F32, tag="cs", name="cs")[:, :w]
                        pend = None  # (m0, m1) awaiting colsum
                        for j in range(NPAIR):
                            if j == 2 and pending_rep is not None:
                                emit_rep_chain(*pending_rep)
                                pending_rep = None
                            tp = tps.tile([128, 2, 512], F32, tag="t", name="t")
                            m0, m1 = 2 * j, 2 * j + 1
                            nc.tensor.matmul(
                                tp[:, 0, :w],
                                lhsT=kk_sb[0:64, m0 * 128:(m0 + 1) * 128],
                                rhs=qq_sb[0:64, lo:lo + w],
                                start=True, stop=True, tile_position=(0, 0))
                            nc.tensor.matmul(
                                tp[:, 1, :w],
                                lhsT=kk_sb[64:128, m1 * 128:(m1 + 1) * 128],
                                rhs=qq_sb[64:128, lo:lo + w],
                                start=True, stop=True, tile_position=(64, 0))
                            nc.scalar.activation(
                                out=expt_sb[:, m0:m0 + 2, lo:lo + w],
                                in_=tp[:, :, :w], func=AF.Exp, bias=shift)
                            if step % 5 in (1, 3):
                                maybe_b()
                            if pend is not None:
                                for m in pend:
                                    nc.tensor.matmul(
                                        cs, lhsT=onec,
                                        rhs=expt_sb[:, m, lo:lo + w],
                                        start=(m == 0), stop=(m == MC - 1))
                            pend = (m0, m1)
                            step += 1
                        maybe_b()
                        for m in pend:
                            nc.tensor.matmul(
                                cs, lhsT=onec, rhs=expt_sb[:, m, lo:lo + w],
                                start=(m == 0), stop=(m == MC - 1))
                        pending_rep = (lo, w, cs)
                    while b_next[0] < MC:
                        maybe_b()
                    emit_rep_chain(*pending_rep)

                # ---- Phase E: out[c, n] = vT.T @ expT; epilogue + residual ----
                with tc.tile_pool(name="out_ps", bufs=1, space="PSUM") as ops:
                    for ci in range(CI):
                        pss = [
                            ops.tile([128, 512], F32, tag=f"out{k}", name=f"out{k}")[:, :w]
                            for k, (lo, w) in enumerate(N_BLOCKS)
                        ]
                        for m2 in range(MC // 2):
                            for k, (lo, w) in enumerate(N_BLOCKS):
                                nc.tensor.matmul(
                                    pss[k],
                                    lhsT=vt_sb[:, 2 * m2:2 * m2 + 2,
                                               ci * 128:(ci + 1) * 128],
                                    rhs=expt_sb[:, 2 * m2:2 * m2 + 2, lo:lo + w],
                                    start=(m2 == 0), stop=(m2 == MC // 2 - 1),
                                    perf_mode=mybir.MatmulPerfMode.DoubleRow)
                        for k, (lo, w) in enumerate(N_BLOCKS):
                            y = ytile.tile([128, 512], F32, tag="y", name="y")[:, :w]
                            nc.vector.tensor_mul(
                                out=y, in0=pss[k], in1=rep_sb[:, lo:lo + w])
                            nc.vector.tensor_add(
                                out=y, in0=y, in1=x_sb[:, ci, lo:lo + w])
                            nc.sync.dma_start(
                                out=out_d[b, ci * 128:(ci + 1) * 128, lo:lo + w], in_=y)
    split_multi_waits(nc)
    return nc


_NC_CACHE = {}


def _get_nc():
    if "nc" not in _NC_CACHE:
        _NC_CACHE["nc"] = build_bass()
    return _NC_CACHE["nc"]


def _prepare_in_maps(feat_map, Wq, bq, Wk, bk, Wv, bv, gamma):
    feat_map = np.asarray(feat_map, dtype=np.float32)
    Wq = np.asarray(Wq, dtype=np.float32)
    bq = np.asarray(bq, dtype=np.float32)
    Wk = np.asarray(Wk, dtype=np.float32)
    bk = np.asarray(bk, dtype=np.float32)
    Wv = np.asarray(Wv, dtype=np.float32)
    bv = np.asarray(bv, dtype=np.float32)
    gamma = np.asarray(gamma, dtype=np.float32)

    bf = ml_dtypes.bfloat16
    # x: [B, C, HW] -> [B, 128, CI, HW] (partition = c % 128, chunk = c // 128)
    x = feat_map.reshape(B, CI, 128, HW).transpose(0, 2, 1, 3).astype(bf)
    x = np.ascontiguousarray(x)

    def prep_w(Wt2):  # [C, M] -> [128, CI, M]
        M = Wt2.shape[1]
        return np.ascontiguousarray(
            Wt2.reshape(CI, 128, M).transpose(1, 0, 2).astype(bf))

    wqq = prep_w(np.concatenate([Wq.T, Wq.T], axis=1))   # [512, 128]
    wkk = prep_w(np.concatenate([Wk.T, Wk.T], axis=1))
    wv = prep_w(Wv.T)                                     # [512, 512]
    bqq = np.concatenate([bq, bq]).reshape(128, 1).astype(np.float32)
    bkk = np.concatenate([bk, bk]).reshape(128, 1).astype(np.float32)
    bvr = np.ascontiguousarray(np.tile(bv[None, :], (128, 1)).astype(np.float32))
    onec = np.ones((128, 1), dtype=mybir.dt.np(FP8))
    oner = np.ones((1, 128), dtype=bf)
    gam = gamma.reshape(1, 1).astype(np.float32)

    in_maps = []
    for i in range(NCORES):
        in_maps.append({
            "x": np.ascontiguousarray(x[i * BPC:(i + 1) * BPC]),
            "wqq": wqq, "wkk": wkk, "wv": wv,
            "bqq": bqq, "bkk": bkk, "bvr": bvr,
            "onec": onec, "oner": oner, "gamma": gam,
        })
    return in_maps


def _run(in_maps, trace=False, **kw):
    nc = _get_nc()
    res = run_bass_kernel_spmd(nc, in_maps, core_ids=list(range(NCORES)),
                               trace=trace, **kw)
    out = np.concatenate([res.results[i]["out"] for i in range(NCORES)], axis=0)
    return out.reshape(B, C, H, W).astype(np.float32), res


def kernel(feat_map, Wq, bq, Wk, bk, Wv, bv, gamma):
    in_maps = _prepare_in_maps(feat_map, Wq, bq, Wk, bk, Wv, bv, gamma)
    out, _ = _run(in_maps, trace=False)
    return out
